# revision 18
# baseline (speedup 1.0000x reference)
"""Distributed Trainium2 kernel: relative-position multi-head attention.

B=2, N=2048, DIM=1536, H=8, DK=64, DV=192.

Sharding: one head per core, both batches (8 heads / 8 cores).  Each core
projects q/k/v for its head over all 4096 tokens, runs attention, transposes
its attention output to feature-major, then one 8-core AllToAll exchanges
token-slices: core c ends up with all 8 heads' outputs for flat token rows
[c*512,(c+1)*512) of the flattened [4096, DIM] output, and computes that
slice of the output projection.

Math: rel_k = distances @ W_rel is rank-1, so after relative_shift the
positional logits are s_i*(j-i) with s_i = (q_i*scale+rpb)@w_h.  The -s_i*i
term is constant per softmax row and drops under softmax.  So
logits = (q*scale+rcb)@k^T + s_i*j, realized as 2 extra contraction rows of
the QK^T matmul: [jvec, ones] on the K side and [s_i, -B_i] on the Q side,
where B_i upper-bounds the row max so exp cannot overflow; it cancels
exactly in softmax.  s_i comes free out of the projection via an extra
weight column u = Wq_scaled @ w_h.
"""

import contextlib

import ml_dtypes
import numpy as np

import concourse.bass as bass
import concourse.bacc as bacc_mod
import concourse.mybir as mybir
import concourse.tile as tile
from concourse.bass_utils import run_bass_kernel_spmd
from concourse.masks import make_identity

B, N, DIM, H, DK, DV = 2, 2048, 1536, 8, 64, 192
NCORES = 8
NT = B * N               # 4096 flat tokens
NQ = NT // NCORES        # 512 output rows per core
P = 128
DCH = DIM // P           # 12 projection contraction chunks
NTILE = N // P           # 16 token tiles per batch
IBLK = 512
NIB = N // IBLK          # 4 i-blocks per batch
F32 = mybir.dt.float32
F32R = mybir.dt.float32r
BF16 = mybir.dt.bfloat16
AT = mybir.AluOpType
AF = mybir.ActivationFunctionType
CONTENT_BOUND = 48.0

_CACHE = {}


def _build_body(nc, tc, xT, wqk, wv1, wv2, wo, krows, ccol, bor, out):
    ctx = contextlib.ExitStack()
    with ctx:
        persist = ctx.enter_context(tc.tile_pool(name="persist", bufs=1))

        wqk_sb = persist.tile([P, DCH * P], BF16, tag="wqk")
        wv1_sb = persist.tile([P, DCH * P], BF16, tag="wv1")
        wv2_sb = persist.tile([P, DCH * 65], BF16, tag="wv2")
        wo_sb = persist.tile([P, DCH * DIM], BF16, tag="wo")
        ccol_sb = persist.tile([P, 2], F32, tag="ccol")
        bor_sb = persist.tile([P, DIM], F32, tag="bor")
        ident = persist.tile([P, P], BF16, tag="ident")

        for w_sb, w_dram in ((wqk_sb, wqk), (wv1_sb, wv1), (wv2_sb, wv2), (wo_sb, wo)):
            nc.sync.dma_start(
                out=w_sb[:].rearrange("p (c m) -> p c m", c=DCH),
                in_=w_dram[:].rearrange("(c p) m -> p c m", p=P))
        nc.sync.dma_start(out=ccol_sb[:], in_=ccol[:])
        nc.sync.dma_start(out=bor_sb[:], in_=bor[:])
        make_identity(nc, ident[:])

        # Q'/K' per flat token: rows 0-63 content, 64 ramp (s / jvec), 65 bias (-B / ones)
        QT = persist.tile([67, NT], BF16, tag="QT")
        KT = persist.tile([67, NT], BF16, tag="KT")
        # v token-major per (b, j-tile): [dv(192) | ones]
        vtok = persist.tile([P, B * NTILE * (DV + 1)], BF16, tag="vtok")
        # attention output feature-major (transposed), split 128/64 partitions
        attTa = persist.tile([P, NT], BF16, tag="attTa")
        attTb = persist.tile([64, NT], BF16, tag="attTb")

        for b in range(B):
            bs = b * N
            nc.sync.dma_start(out=KT[64:67, bs:bs + N], in_=krows[:])  # jhi; jlo; ones

        # ---------------- phase 1: projections ----------------
        with tc.tile_pool(name="xch", bufs=4) as xpool, \
             tc.tile_pool(name="prjp", bufs=2, space="PSUM") as ppsum, \
             tc.tile_pool(name="vT", bufs=1) as vtpool, \
             tc.tile_pool(name="vtp", bufs=2, space="PSUM") as vtpsum, \
             tc.tile_pool(name="srow", bufs=1) as spool, \
             tc.tile_pool(name="sdram", bufs=2, space="DRAM") as sdram:

            srow = [spool.tile([1, N], F32, tag=f"srow{b}", name=f"srow{b}") for b in range(B)]

            vTa = vtpool.tile([P, NT], BF16, tag="vTa")
            vTb = vtpool.tile([64, NT], BF16, tag="vTb")

            for bt in range(B * NIB):
                sl = slice(bt * IBLK, (bt + 1) * IBLK)
                pqk = ppsum.tile([P, IBLK], F32, tag="pqk")
                pv1 = ppsum.tile([P, IBLK], F32, tag="pv1")
                pv2 = ppsum.tile([P, IBLK], F32, tag="pv2")
                for c in range(DCH):
                    xc = xpool.tile([P, IBLK], BF16, tag="xc")
                    nc.gpsimd.dma_start(out=xc[:], in_=xT[c * P:(c + 1) * P, sl])
                    mk = dict(start=(c == 0), stop=(c == DCH - 1))
                    xr = xc[:]
                    nc.tensor.matmul(pqk[:], wqk_sb[:, c * P:(c + 1) * P], xr, **mk)
                    nc.tensor.matmul(pv1[:], wv1_sb[:, c * P:(c + 1) * P], xr, **mk)
                    nc.tensor.matmul(pv2[0:65, :], wv2_sb[:, c * 65:(c + 1) * 65], xr, **mk)
                # q rows: + rcb;  k rows: copy;  v rows -> bf16 feature-major
                nc.vector.tensor_scalar_add(QT[0:DK, sl], pqk[0:DK, :], ccol_sb[0:DK, 0:1])
                nc.vector.tensor_copy(KT[0:DK, sl], pqk[DK:2 * DK, :])
                nc.vector.tensor_copy(vTa[:, sl], pv1[:])
                nc.vector.tensor_copy(vTb[:, sl], pv2[0:DK, :])
                # s = x@u + C_h  (staged at partition 0; moved to QT row 64 later)
                nc.vector.tensor_scalar_add(
                    srow[bt // NIB][0:1, (bt % NIB) * IBLK:(bt % NIB + 1) * IBLK],
                    pv2[64:65, :], ccol_sb[0:1, 1:2])

            # v -> token-major packed [dv(192) | ones] per (b, j-tile)
            for bj in range(B * NTILE):
                tp = vtpsum.tile([P, 2 * P], BF16, tag="vtp")
                nc.tensor.transpose(tp[:, 0:P], vTa[:, bj * P:(bj + 1) * P], ident[:])
                nc.tensor.transpose(tp[:, P:P + 64], vTb[:, bj * P:(bj + 1) * P], ident[0:64, 0:64])
                base = bj * (DV + 1)
                nc.vector.tensor_copy(vtok[:, base:base + P], tp[:, 0:P])
                nc.vector.tensor_copy(vtok[:, base + P:base + DV], tp[:, P:P + 64])
                nc.gpsimd.memset(vtok[:, base + DV:base + DV + 1], 1.0)

            # ramp/bias rows [s8; s1; -B] into QT[64:67] via DRAM bounce
            # B = relu(s)*(N-1) + CONTENT_BOUND  (ramp max over j; -s*i dropped)
            for b in range(B):
                bs = b * N
                t8 = spool.tile([1, N], BF16, tag="bt8")
                t1f = spool.tile([1, N], BF16, tag="bt1f")
                tb = spool.tile([1, N], BF16, tag="btb")
                tr = spool.tile([1, N], F32, tag="btr")
                nc.vector.tensor_scalar_mul(t8[:], srow[b][:], 8.0)
                nc.vector.tensor_copy(t1f[:], srow[b][:])
                nc.vector.tensor_scalar_max(tr[:], srow[b][:], 0.0)
                nc.vector.tensor_scalar(tb[:], tr[:], -float(N - 1), -CONTENT_BOUND, AT.mult, AT.add)
                qs3 = sdram.tile([3, N], BF16, tag="qs3")
                nc.sync.dma_start(out=qs3[0:1, :], in_=t8[:])
                nc.sync.dma_start(out=qs3[1:2, :], in_=t1f[:])
                nc.sync.dma_start(out=qs3[2:3, :], in_=tb[:])
                nc.sync.dma_start(out=QT[64:67, bs:bs + N], in_=qs3[:])

        # ---------------- phase 2: attention ----------------
        with tc.tile_pool(name="et", bufs=2) as epool, \
             tc.tile_pool(name="lg", bufs=3, space="PSUM") as lgp, \
             tc.tile_pool(name="av", bufs=2, space="PSUM") as avp, \
             tc.tile_pool(name="atp", bufs=1, space="PSUM") as atpp, \
             tc.tile_pool(name="rz", bufs=4) as rzpool, \
             tc.tile_pool(name="an", bufs=4) as anpool:
            for b in range(B):
                bs = b * N
                for ib in range(NIB):
                    isl = slice(bs + ib * IBLK, bs + (ib + 1) * IBLK)
                    eT = epool.tile([P, NTILE * IBLK], BF16, tag="eT")
                    for jt in range(NTILE):
                        lg = lgp.tile([P, IBLK], F32, tag="lg")
                        nc.tensor.matmul(
                            lg[:], KT[:, bs + jt * P:bs + (jt + 1) * P],
                            QT[:, isl], start=True, stop=True)
                        nc.scalar.activation(eT[:, jt * IBLK:(jt + 1) * IBLK], lg[:], AF.Exp)
                    for ic in range(IBLK // P):
                        av = avp.tile([P, DV + 1], F32, tag="av")
                        for jt in range(NTILE):
                            nc.tensor.matmul(
                                av[:],
                                eT[:, jt * IBLK + ic * P:jt * IBLK + (ic + 1) * P],
                                vtok[:, (b * NTILE + jt) * (DV + 1):(b * NTILE + jt + 1) * (DV + 1)],
                                start=(jt == 0), stop=(jt == NTILE - 1))
                        rz = rzpool.tile([P, 1], F32, tag="rz")
                        nc.vector.reciprocal(rz[:], av[:, DV:DV + 1])
                        an = anpool.tile([P, DV], BF16, tag="an")
                        nc.vector.tensor_scalar_mul(an[:], av[:, 0:DV], rz[:])
                        # transpose [i, dv] -> feature-major attTa/attTb
                        iabs = bs + ib * IBLK + ic * P
                        ta = atpp.tile([P, P], BF16, tag="ta")
                        nc.tensor.transpose(ta[:], an[:, 0:P], ident[:])
                        nc.vector.tensor_copy(attTa[:, iabs:iabs + P], ta[:])
                        tb = atpp.tile([P, P], BF16, tag="tb")
                        nc.tensor.transpose(tb[0:64, 0:P], an[:, P:DV], ident[:])
                        nc.vector.tensor_copy(attTb[:, iabs:iabs + P], tb[0:64, 0:P])

        # ---------------- phase 3: AllToAll + output projection ----------------
        with tc.tile_pool(name="dram", bufs=1, space="DRAM") as dram, \
             tc.tile_pool(name="gat", bufs=1) as gpool, \
             tc.tile_pool(name="yp", bufs=4, space="PSUM") as ypsum, \
             tc.tile_pool(name="yo", bufs=4) as ypool:
            a2a_in = dram.tile([NCORES * DV, NQ], BF16)    # [dest, dv, i-slice]
            a2a_out = dram.tile([NCORES * DV, NQ], BF16)   # [src, dv, my-slice] = [1536, 512]
            nc.sync.dma_start(
                out=a2a_in[:].rearrange("(d r) n -> r d n", r=DV)[0:P],
                in_=attTa[:].rearrange("p (d n) -> p d n", d=NCORES))
            nc.sync.dma_start(
                out=a2a_in[:].rearrange("(d r) n -> r d n", r=DV)[P:DV],
                in_=attTb[:].rearrange("p (d n) -> p d n", d=NCORES))
            nc.gpsimd.collective_compute(
                "AllToAll", AT.bypass,
                replica_groups=[list(range(NCORES))],
                ins=[a2a_in[:].opt()], outs=[a2a_out[:].opt()])
            gatT = gpool.tile([P, DCH * NQ], BF16, tag="gatT")
            nc.sync.dma_start(
                out=gatT[:].rearrange("p (c n) -> p c n", c=DCH),
                in_=a2a_out[:].rearrange("(c p) n -> p c n", p=P))
            for it in range(NQ // P):
                for ot in range(DIM // IBLK):
                    yp = ypsum.tile([P, IBLK], F32, tag="yp")
                    for kc in range(DCH):
                        nc.tensor.matmul(
                            yp[:], gatT[:, kc * NQ + it * P:kc * NQ + (it + 1) * P],
                            wo_sb[:, kc * DIM + ot * IBLK:kc * DIM + (ot + 1) * IBLK],
                            start=(kc == 0), stop=(kc == DCH - 1))
                    yo = ypool.tile([P, IBLK], F32, tag="yo")
                    nc.vector.tensor_add(yo[:], yp[:], bor_sb[:, ot * IBLK:(ot + 1) * IBLK])
                    nc.sync.dma_start(
                        out=out[it * P:(it + 1) * P, ot * IBLK:(ot + 1) * IBLK], in_=yo[:])


def build_nc():
    nc = bacc_mod.Bacc(None, target_bir_lowering=False, debug=False)
    xT = nc.declare_dram_parameter("xT", [DIM, NT], BF16, isOutput=False)
    wqk = nc.declare_dram_parameter("wqk", [DIM, P], BF16, isOutput=False)
    wv1 = nc.declare_dram_parameter("wv1", [DIM, P], BF16, isOutput=False)
    wv2 = nc.declare_dram_parameter("wv2", [DIM, 65], BF16, isOutput=False)
    wo = nc.declare_dram_parameter("wo", [DIM, DIM], BF16, isOutput=False)
    krows = nc.declare_dram_parameter("krows", [3, N], BF16, isOutput=False)
    ccol = nc.declare_dram_parameter("ccol", [P, 2], F32, isOutput=False)
    bor = nc.declare_dram_parameter("bor", [P, DIM], F32, isOutput=False)
    out = nc.declare_dram_parameter("out", [NQ, DIM], F32, isOutput=True)
    with tile.TileContext(nc) as tc:
        _build_body(nc, tc, xT, wqk, wv1, wv2, wo, krows, ccol, bor, out)
    nc.compile()
    return nc


def _in_maps(x, Wq, Wk, Wv, W_rel, Wo, bo, rcb, rpb):
    scale = np.float32(DK ** -0.5)
    Wq_s = (Wq * scale).astype(np.float32)
    iota = np.arange(N, dtype=np.float32)
    jhi = np.floor(iota / 8)
    jlo = iota - 8 * jhi
    krows = np.stack([jhi, jlo, np.ones(N, np.float32)]).astype(ml_dtypes.bfloat16)
    wo_bf = Wo.astype(ml_dtypes.bfloat16)
    bor = np.broadcast_to(bo.astype(np.float32), (P, DIM)).copy()
    xTb = np.ascontiguousarray(np.concatenate([x[0].T, x[1].T], axis=1)).astype(ml_dtypes.bfloat16)
    maps = []
    for h in range(NCORES):
        qs, ks = Wq_s[:, h * DK:(h + 1) * DK], Wk[:, h * DK:(h + 1) * DK]
        vs = Wv[:, h * DV:(h + 1) * DV]
        w_h = W_rel[0, h * DK:(h + 1) * DK]
        u = qs @ w_h                                  # [DIM]
        wv2 = np.concatenate([vs[:, P:DV], u[:, None]], axis=1)  # [DIM, 65]
        ccol = np.zeros((P, 2), np.float32)
        ccol[0:DK, 0] = rcb[h]
        ccol[0, 1] = float(rpb[h] @ w_h)              # C_h
        maps.append({
            "xT": xTb,
            "wqk": np.ascontiguousarray(np.concatenate([qs, ks], axis=1)).astype(ml_dtypes.bfloat16),
            "wv1": np.ascontiguousarray(vs[:, 0:P]).astype(ml_dtypes.bfloat16),
            "wv2": np.ascontiguousarray(wv2).astype(ml_dtypes.bfloat16),
            "wo": wo_bf,
            "krows": krows,
            "ccol": ccol,
            "bor": bor,
        })
    return maps


def kernel(x, Wq, Wk, Wv, W_rel, Wo, bo, rel_content_bias, rel_pos_bias):
    x = np.asarray(x, np.float32)
    rcb = np.asarray(rel_content_bias, np.float32)[0, :, 0, :]
    rpb = np.asarray(rel_pos_bias, np.float32)[0, :, 0, :]
    if "nc" not in _CACHE:
        _CACHE["nc"] = build_nc()
    nc = _CACHE["nc"]
    maps = _in_maps(x, np.asarray(Wq, np.float32), np.asarray(Wk, np.float32),
                    np.asarray(Wv, np.float32), np.asarray(W_rel, np.float32),
                    np.asarray(Wo, np.float32), np.asarray(bo, np.float32), rcb, rpb)
    res = run_bass_kernel_spmd(nc, maps, core_ids=list(range(NCORES)))
    out = np.zeros((B * N, DIM), np.float32)
    for c in range(NCORES):
        out[c * NQ:(c + 1) * NQ, :] = res.results[c]["out"]
    return out.reshape(B, N, DIM)


# revision 25
# speedup vs baseline: 1.2176x; 1.2176x over previous
"""Distributed Trainium2 kernel: relative-position multi-head attention.

B=2, N=2048, DIM=1536, H=8, DK=64, DV=192.

Sharding: one head per core, both batches (8 heads / 8 cores).  Each core
projects q/k/v for its head over all 4096 tokens, runs attention, transposes
its attention output to feature-major, then one 8-core AllToAll exchanges
token-slices: core c ends up with all 8 heads' outputs for flat token rows
[c*512,(c+1)*512) of the flattened [4096, DIM] output, and computes that
slice of the output projection.

Math: rel_k = distances @ W_rel is rank-1, so after relative_shift the
positional logits are s_i*(j-i) with s_i = (q_i*scale+rpb)@w_h.  The -s_i*i
term is constant per softmax row and drops under softmax.  So
logits = (q*scale+rcb)@k^T + s_i*j, realized as 2 extra contraction rows of
the QK^T matmul: [jvec, ones] on the K side and [s_i, -B_i] on the Q side,
where B_i upper-bounds the row max so exp cannot overflow; it cancels
exactly in softmax.  s_i comes free out of the projection via an extra
weight column u = Wq_scaled @ w_h.
"""

import contextlib

import ml_dtypes
import numpy as np

import concourse.bass as bass
import concourse.bacc as bacc_mod
import concourse.mybir as mybir
import concourse.tile as tile
from concourse.bass_utils import run_bass_kernel_spmd
from concourse.masks import make_identity

B, N, DIM, H, DK, DV = 2, 2048, 1536, 8, 64, 192
NCORES = 8
NT = B * N               # 4096 flat tokens
NQ = NT // NCORES        # 512 output rows per core
P = 128
DCH = DIM // P           # 12 projection contraction chunks
NTILE = N // P           # 16 token tiles per batch
IBLK = 512
NIB = N // IBLK          # 4 i-blocks per batch
F32 = mybir.dt.float32
F32R = mybir.dt.float32r
BF16 = mybir.dt.bfloat16
AT = mybir.AluOpType
AF = mybir.ActivationFunctionType
CONTENT_BOUND = 48.0

_CACHE = {}


def _build_body(nc, tc, xT, wqk, wv1, wv2, wo, krows, ccol, bor, out):
    ctx = contextlib.ExitStack()
    with ctx:
        persist = ctx.enter_context(tc.tile_pool(name="persist", bufs=1))

        wqk_sb = persist.tile([P, DCH * P], BF16, tag="wqk")
        wv1_sb = persist.tile([P, DCH * P], BF16, tag="wv1")
        wv2_sb = persist.tile([P, DCH * 65], BF16, tag="wv2")
        wo_sb = persist.tile([P, DCH * DIM], BF16, tag="wo")
        ccol_sb = persist.tile([P, 2], F32, tag="ccol")
        bor_sb = persist.tile([P, DIM], F32, tag="bor")
        ident = persist.tile([P, P], BF16, tag="ident")

        for w_sb, w_dram in ((wqk_sb, wqk), (wv1_sb, wv1), (wv2_sb, wv2)):
            nc.sync.dma_start(out=w_sb[:], in_=w_dram[:])
        nc.sync.dma_start(out=ccol_sb[:], in_=ccol[:])
        make_identity(nc, ident[:])

        # Q'/K' per flat token: rows 0-63 content, 64 ramp (s / jvec), 65 bias (-B / ones)
        QT = persist.tile([67, NT], BF16, tag="QT")
        KT = persist.tile([67, NT], BF16, tag="KT")
        # v token-major per (b, j-tile): [dv(192) | ones]
        vtok = persist.tile([P, B * NTILE * (DV + 1)], BF16, tag="vtok")
        # attention output feature-major (transposed), split 128/64 partitions
        attTa = persist.tile([P, NT], BF16, tag="attTa")
        attTb = persist.tile([64, NT], BF16, tag="attTb")

        for b in range(B):
            bs = b * N
            nc.sync.dma_start(out=KT[64:67, bs:bs + N], in_=krows[:])  # jhi; jlo; ones

        # ---------------- phase 1: projections ----------------
        with tc.tile_pool(name="xch", bufs=8) as xpool, \
             tc.tile_pool(name="prjp", bufs=2, space="PSUM") as ppsum, \
             tc.tile_pool(name="vT", bufs=1) as vtpool, \
             tc.tile_pool(name="vtp", bufs=2, space="PSUM") as vtpsum, \
             tc.tile_pool(name="srow", bufs=1) as spool, \
             tc.tile_pool(name="sdram", bufs=2, space="DRAM") as sdram:

            srow = [spool.tile([1, N], F32, tag=f"srow{b}", name=f"srow{b}") for b in range(B)]

            vTa = vtpool.tile([P, NT], BF16, tag="vTa")
            vTb = vtpool.tile([64, NT], BF16, tag="vTb")

            xc_cache = {}
            for b in range(B):
                for it in range(NIB):
                    bt = b * NIB + it
                    sl = slice(bt * IBLK, (bt + 1) * IBLK)
                    pqk = ppsum.tile([P, IBLK], F32, tag="pqk", name=f"pqk{bt}")
                    pv1 = ppsum.tile([P, IBLK], F32, tag="pv1", name=f"pv1{bt}")
                    pv2 = ppsum.tile([P, IBLK], F32, tag="pv2", name=f"pv2{bt}")
                    for c in range(DCH):
                        if it % 2 == 0:
                            xc2 = xpool.tile([P, 2 * IBLK], BF16, tag="xc", name=f"xc{bt}_{c}")
                            nc.sync.dma_start(
                                out=xc2[:], in_=xT[c * P:(c + 1) * P, bt * IBLK:(bt + 2) * IBLK])
                            xc_cache[c] = xc2
                        mk = dict(start=(c == 0), stop=(c == DCH - 1))
                        xr = xc_cache[c][:, (it % 2) * IBLK:(it % 2 + 1) * IBLK]
                        nc.tensor.matmul(pqk[:], wqk_sb[:, c * P:(c + 1) * P], xr, **mk)
                        nc.tensor.matmul(pv1[:], wv1_sb[:, c * P:(c + 1) * P], xr, **mk)
                        nc.tensor.matmul(pv2[0:65, :], wv2_sb[:, c * 65:(c + 1) * 65], xr, **mk)
                    nc.vector.tensor_scalar_add(QT[0:DK, sl], pqk[0:DK, :], ccol_sb[0:DK, 0:1])
                    nc.vector.tensor_copy(KT[0:DK, sl], pqk[DK:2 * DK, :])
                    nc.vector.tensor_copy(vTa[:, sl], pv1[:])
                    nc.vector.tensor_copy(vTb[:, sl], pv2[0:DK, :])
                    nc.vector.tensor_scalar_add(
                        srow[b][0:1, it * IBLK:(it + 1) * IBLK],
                        pv2[64:65, :], ccol_sb[0:1, 1:2])

                # assemble batch b immediately: v token-major + QT ramp rows
                bs = b * N
                for jt in range(NTILE):
                    bj = b * NTILE + jt
                    tp = vtpsum.tile([P, 2 * P], BF16, tag="vtp", name=f"vtp{bj}")
                    nc.tensor.transpose(tp[:, 0:P], vTa[:, bj * P:(bj + 1) * P], ident[:])
                    nc.tensor.transpose(tp[:, P:P + 64], vTb[:, bj * P:(bj + 1) * P], ident[0:64, 0:64])
                    base = bj * (DV + 1)
                    nc.vector.tensor_copy(vtok[:, base:base + P], tp[:, 0:P])
                    nc.vector.tensor_copy(vtok[:, base + P:base + DV], tp[:, P:P + 64])
                    nc.gpsimd.memset(vtok[:, base + DV:base + DV + 1], 1.0)
                # B = relu(s)*(N-1) + CONTENT_BOUND  (ramp max over j; -s*i dropped)
                t8 = spool.tile([1, N], BF16, tag="bt8", name=f"bt8{b}")
                t1f = spool.tile([1, N], BF16, tag="bt1f", name=f"bt1f{b}")
                tb = spool.tile([1, N], BF16, tag="btb", name=f"btb{b}")
                tr = spool.tile([1, N], F32, tag="btr", name=f"btr{b}")
                nc.vector.tensor_scalar_mul(t8[:], srow[b][:], 8.0)
                nc.vector.tensor_copy(t1f[:], srow[b][:])
                nc.vector.tensor_scalar_max(tr[:], srow[b][:], 0.0)
                nc.vector.tensor_scalar(tb[:], tr[:], -float(N - 1), -CONTENT_BOUND, AT.mult, AT.add)
                qs3 = sdram.tile([3, N], BF16, tag="qs3", name=f"qs3{b}")
                nc.sync.dma_start(out=qs3[0:1, :], in_=t8[:])
                nc.sync.dma_start(out=qs3[1:2, :], in_=t1f[:])
                nc.sync.dma_start(out=qs3[2:3, :], in_=tb[:])
                nc.sync.dma_start(out=QT[64:67, bs:bs + N], in_=qs3[:])

        # ---------------- phase 2: attention ----------------
        with tc.tile_pool(name="et", bufs=2) as epool, \
             tc.tile_pool(name="lg", bufs=3, space="PSUM") as lgp, \
             tc.tile_pool(name="av", bufs=2, space="PSUM") as avp, \
             tc.tile_pool(name="atp", bufs=1, space="PSUM") as atpp, \
             tc.tile_pool(name="rz", bufs=4) as rzpool, \
             tc.tile_pool(name="an", bufs=4) as anpool:
            for b in range(B):
                bs = b * N
                for ib in range(NIB):
                    isl = slice(bs + ib * IBLK, bs + (ib + 1) * IBLK)
                    eT = epool.tile([P, NTILE * IBLK], BF16, tag="eT")
                    for jt in range(NTILE):
                        lg = lgp.tile([P, IBLK], F32, tag="lg")
                        nc.tensor.matmul(
                            lg[:], KT[:, bs + jt * P:bs + (jt + 1) * P],
                            QT[:, isl], start=True, stop=True)
                        nc.scalar.activation(eT[:, jt * IBLK:(jt + 1) * IBLK], lg[:], AF.Exp)
                    for ic in range(IBLK // P):
                        av = avp.tile([P, DV + 1], F32, tag="av")
                        for jt in range(NTILE):
                            nc.tensor.matmul(
                                av[:],
                                eT[:, jt * IBLK + ic * P:jt * IBLK + (ic + 1) * P],
                                vtok[:, (b * NTILE + jt) * (DV + 1):(b * NTILE + jt + 1) * (DV + 1)],
                                start=(jt == 0), stop=(jt == NTILE - 1))
                        rz = rzpool.tile([P, 1], F32, tag="rz")
                        nc.vector.reciprocal(rz[:], av[:, DV:DV + 1])
                        an = anpool.tile([P, DV], BF16, tag="an")
                        nc.vector.tensor_scalar_mul(an[:], av[:, 0:DV], rz[:])
                        # transpose [i, dv] -> feature-major attTa/attTb
                        iabs = bs + ib * IBLK + ic * P
                        ta = atpp.tile([P, P], BF16, tag="ta")
                        nc.tensor.transpose(ta[:], an[:, 0:P], ident[:])
                        nc.vector.tensor_copy(attTa[:, iabs:iabs + P], ta[:])
                        tb = atpp.tile([P, P], BF16, tag="tb")
                        nc.tensor.transpose(tb[0:64, 0:P], an[:, P:DV], ident[:])
                        nc.vector.tensor_copy(attTb[:, iabs:iabs + P], tb[0:64, 0:P])

        # ---------------- phase 3: per-batch AllToAll + output projection ----------------
        # a2a[b]: input [8 dest, 192 dv, 256 rows of batch b] -> core c owns
        # batch-b rows [c*256,(c+1)*256).  out rows: [b0 256 | b1 256].
        HQ = NQ // B  # 256 rows per batch per core
        with tc.tile_pool(name="dram", bufs=1, space="DRAM") as dram, \
             tc.tile_pool(name="gat", bufs=1) as gpool, \
             tc.tile_pool(name="yp", bufs=4, space="PSUM") as ypsum, \
             tc.tile_pool(name="yo", bufs=4) as ypool:
            gatT = gpool.tile([P, DCH * NQ], BF16, tag="gatT")
            nc.sync.dma_start(out=wo_sb[:], in_=wo[:])
            nc.sync.dma_start(out=bor_sb[:], in_=bor[:])

            def do_exchange(b):
                bs = b * N
                a2a_in = dram.tile([NCORES * DV, HQ], BF16, name=f"a2a_in{b}", tag=f"a2a_in{b}")
                a2a_out = dram.tile([NCORES * DV, HQ], BF16, name=f"a2a_out{b}", tag=f"a2a_out{b}")
                nc.sync.dma_start(
                    out=a2a_in[:].rearrange("(d r) n -> r d n", r=DV)[0:P],
                    in_=attTa[:, bs:bs + N].rearrange("p (d n) -> p d n", d=NCORES))
                nc.sync.dma_start(
                    out=a2a_in[:].rearrange("(d r) n -> r d n", r=DV)[P:DV],
                    in_=attTb[:, bs:bs + N].rearrange("p (d n) -> p d n", d=NCORES))
                nc.gpsimd.collective_compute(
                    "AllToAll", AT.bypass,
                    replica_groups=[list(range(NCORES))],
                    ins=[a2a_in[:].opt()], outs=[a2a_out[:].opt()])
                # received [src, dv 192, my 256 rows] = [1536 dv, 256]
                nc.sync.dma_start(
                    out=gatT[:].rearrange("p (c n) -> p c n", c=DCH)[:, :, b * HQ:(b + 1) * HQ],
                    in_=a2a_out[:].rearrange("(c p) n -> p c n", p=P))

            def do_outproj(b, ypsum, ypool):
                for it in range(HQ // P):
                    row = b * HQ + it * P
                    for ot in range(DIM // IBLK):
                        yp = ypsum.tile([P, IBLK], F32, tag="yp")
                        for kc in range(DCH):
                            nc.tensor.matmul(
                                yp[:], gatT[:, kc * NQ + row:kc * NQ + row + P],
                                wo_sb[:, kc * DIM + ot * IBLK:kc * DIM + (ot + 1) * IBLK],
                                start=(kc == 0), stop=(kc == DCH - 1))
                        yo = ypool.tile([P, IBLK], F32, tag="yo")
                        nc.vector.tensor_add(yo[:], yp[:], bor_sb[:, ot * IBLK:(ot + 1) * IBLK])
                        nc.sync.dma_start(
                            out=out[row:row + P, ot * IBLK:(ot + 1) * IBLK], in_=yo[:])

            for b in range(B):
                do_exchange(b)
                do_outproj(b)
